# revision 3
# baseline (speedup 1.0000x reference)
"""Trainium2 Bass kernel for the soft-decision-tree ensemble classifier.

Math (per batch row b, tree t):
  zb[t,n]      = x[b] . W[t,n] + bias[t,n]
  log s        = zb - softplus(zb);  log(1-s) = -softplus(zb)
  log_leaf[l]  = sum_{k in path(l)} dir_k * zb_k  -  sum_{k in path(l)} softplus(zb_k)
  leaf_prob    = exp(log_leaf)
  out[b,c]     = sum_t 2*softmax(tw)_t * sum_l leaf_prob[t,l] * softmax(leaf_logits[t,l])_c

Mapping: data-parallel over the batch (B=4096 -> 512 rows per NeuronCore).
Per core, logits live in [tree-node (padded 64/tree), batch] layout so the
per-tree path sums become 128-wide matmuls with +/-1 constant matrices
(block-diagonal over a pair of trees per 128-partition tile). All matmuls
run as float32r (full PE rate at free-dim 512). softplus = Ln(Exp(x)+1) so
the whole kernel uses a single ACT function table (exp/ln).
"""

import numpy as np

TREE_DEPTH = 6
T, N, D, C = 64, 63, 512, 100
L = 2**TREE_DEPTH          # 64
NPAD = 64                  # nodes padded per tree
TNP = T * NPAD             # 4096
NTILES = TNP // 128        # 32 (two trees per 128-partition tile)
B = 4096
NCORES = 8
BS = B // NCORES           # 512


def _leaf_paths(depth):
    Ll = 2**depth
    idx = np.zeros((Ll, depth), np.int32)
    dr = np.zeros((Ll, depth), np.int32)
    for l in range(Ll):
        node = 0
        for k in range(depth):
            bit = (l >> (depth - 1 - k)) & 1
            idx[l, k] = node
            dr[l, k] = bit
            node = 2 * node + 1 + bit
    return idx, dr


def _host_consts():
    idx, dr = _leaf_paths(TREE_DEPTH)
    mdir = np.zeros((NPAD, L), np.float32)   # [node, leaf] +1 where dir=1
    mpath = np.zeros((NPAD, L), np.float32)  # [node, leaf] -1 on path
    for l in range(L):
        for k in range(TREE_DEPTH):
            n = idx[l, k]
            mpath[n, l] -= 1.0
            if dr[l, k]:
                mdir[n, l] += 1.0
    adir = np.zeros((128, 128), np.float32)
    apath = np.zeros((128, 128), np.float32)
    adir[:NPAD, :L] = mdir
    adir[NPAD:, L:] = mdir
    apath[:NPAD, :L] = mpath
    apath[NPAD:, L:] = mpath
    # a2[t, p] = 1 if (t % 2) == p // 64 ; e2[t, i] = 1 if t // 2 == i
    a2 = np.zeros((T, 128), np.float32)
    a2[0::2, :64] = 0.0
    for t in range(T):
        a2[t, (t % 2) * 64:(t % 2) * 64 + 64] = 1.0
    e2 = np.zeros((T, NTILES), np.float32)
    for t in range(T):
        e2[t, t // 2] = 1.0
    return adir, apath, a2, e2


_NC_CACHE = {}


def _build_bass():
    import concourse.bacc as bacc
    import concourse.mybir as mybir
    import concourse.tile as tile
    from concourse.masks import make_identity

    dt = mybir.dt
    f32 = dt.float32
    f32r = dt.float32r
    AF = mybir.ActivationFunctionType
    ALU = mybir.AluOpType
    AX = mybir.AxisListType

    nc = bacc.Bacc("TRN2", target_bir_lowering=False, debug=False,
                   num_devices=NCORES)

    xt = nc.dram_tensor("xt", [D, BS], f32r, kind="ExternalInput").ap()
    wt = nc.dram_tensor("wt", [D, TNP], f32r, kind="ExternalInput").ap()
    biasc = nc.dram_tensor("biasc", [128, NTILES], f32, kind="ExternalInput").ap()
    llf = nc.dram_tensor("llf", [TNP, C], f32, kind="ExternalInput").ap()
    tw = nc.dram_tensor("tw", [1, T], f32, kind="ExternalInput").ap()
    adir = nc.dram_tensor("adir", [128, 128], f32r, kind="ExternalInput").ap()
    apath = nc.dram_tensor("apath", [128, 128], f32r, kind="ExternalInput").ap()
    a2 = nc.dram_tensor("a2", [T, 128], f32, kind="ExternalInput").ap()
    e2 = nc.dram_tensor("e2", [T, NTILES], f32, kind="ExternalInput").ap()
    out = nc.dram_tensor("out", [C, BS], f32, kind="ExternalOutput").ap()

    with tile.TileContext(nc) as tc:
        with (
            tc.tile_pool(name="big", bufs=1) as bigp,
            tc.tile_pool(name="const", bufs=1) as constp,
            tc.tile_pool(name="work", bufs=3) as work,
            tc.tile_pool(name="tmp", bufs=2) as tmpp,
            tc.tile_pool(name="ps", bufs=2, space="PSUM") as psp,
            tc.tile_pool(name="ps1", bufs=1, space="PSUM") as ps1,
        ):
            # ---- persistent loads -------------------------------------
            wt_t = []
            for j in range(4):
                wtile = bigp.tile([128, TNP], f32r, tag=f"wt{j}")
                nc.sync.dma_start(out=wtile[:], in_=wt[j * 128:(j + 1) * 128, :])
                wt_t.append(wtile)
            xt_t = []
            for j in range(4):
                xtile = bigp.tile([128, BS], f32r, tag=f"xt{j}")
                nc.sync.dma_start(out=xtile[:], in_=xt[j * 128:(j + 1) * 128, :])
                xt_t.append(xtile)
            ll_t = bigp.tile([128, NTILES * C], f32, tag="ll")
            nc.sync.dma_start(
                out=ll_t[:].rearrange("p (i c) -> p i c", c=C),
                in_=llf.rearrange("(i p) c -> p i c", p=128),
            )
            biasc_t = constp.tile([128, NTILES], f32, tag="biasc")
            nc.sync.dma_start(out=biasc_t[:], in_=biasc[:])
            adir_t = constp.tile([128, 128], f32r, tag="adir")
            nc.sync.dma_start(out=adir_t[:], in_=adir[:])
            apath_t = constp.tile([128, 128], f32r, tag="apath")
            nc.sync.dma_start(out=apath_t[:], in_=apath[:])
            a2_t = constp.tile([T, 128], f32, tag="a2")
            nc.sync.dma_start(out=a2_t[:], in_=a2[:])
            e2_t = constp.tile([T, NTILES], f32, tag="e2")
            nc.sync.dma_start(out=e2_t[:], in_=e2[:])
            tw_t = constp.tile([1, T], f32, tag="tw")
            nc.sync.dma_start(out=tw_t[:], in_=tw[:])

            ident = constp.tile([64, 64], f32, tag="ident")
            make_identity(nc, ident[:])

            # ---- tree-weight softmax -> per-partition scale columns ----
            mneg = constp.tile([1, 1], f32, tag="mneg")
            nc.vector.tensor_reduce(out=mneg[:], in_=tw_t[:], op=ALU.max,
                                    axis=AX.X, negate=True)
            ew = constp.tile([1, T], f32, tag="ew")
            nc.scalar.activation(ew[:], tw_t[:], AF.Exp, bias=mneg[:, 0:1],
                                 scale=1.0)
            sw = constp.tile([1, 1], f32, tag="sw")
            nc.vector.tensor_reduce(out=sw[:], in_=ew[:], op=ALU.add, axis=AX.X)
            rw = constp.tile([1, 1], f32, tag="rw")
            nc.vector.reciprocal(rw[:], sw[:])
            wrow = constp.tile([1, T], f32, tag="wrow")
            nc.vector.tensor_scalar(out=wrow[:], in0=ew[:], scalar1=rw[:, 0:1],
                                    scalar2=2.0, op0=ALU.mult, op1=ALU.mult)
            wcol_ps = ps1.tile([T, 1], f32, tag="wcolps")
            nc.tensor.transpose(wcol_ps[:], wrow[:], ident[0:1, 0:1])
            wcol = constp.tile([T, 1], f32, tag="wcol")
            nc.vector.tensor_copy(out=wcol[:], in_=wcol_ps[:])
            bmat = constp.tile([T, NTILES], f32, tag="bmat")
            nc.vector.tensor_scalar_mul(bmat[:], e2_t[:], wcol[:, 0:1])
            w2_ps = ps1.tile([128, NTILES], f32, tag="w2ps")
            nc.tensor.matmul(w2_ps[:], lhsT=a2_t[:], rhs=bmat[:],
                             start=True, stop=True)
            w2c = constp.tile([128, NTILES], f32, tag="w2c")
            nc.vector.tensor_copy(out=w2c[:], in_=w2_ps[:])

            # ---- main pipeline ---------------------------------------
            out_ps = ps1.tile([C, BS], f32, tag="outps")
            for i in range(NTILES):
                pz = psp.tile([128, BS], f32, tag="pz")
                for j in range(4):
                    nc.tensor.matmul(
                        pz[:],
                        lhsT=wt_t[j][:, i * 128:(i + 1) * 128],
                        rhs=xt_t[j][:],
                        start=(j == 0), stop=(j == 3),
                    )
                bsl = biasc_t[:, i:i + 1]
                ta = work.tile([128, BS], f32r, tag="ta")
                nc.vector.tensor_scalar_add(out=ta[:], in0=pz[:], scalar1=bsl)
                te = tmpp.tile([128, BS], f32, tag="te")
                nc.scalar.activation(te[:], pz[:], AF.Exp, bias=bsl, scale=1.0)
                tb = work.tile([128, BS], f32r, tag="tb")
                nc.scalar.activation(tb[:], te[:], AF.Ln, bias=1.0, scale=1.0)
                pp = psp.tile([128, BS], f32, tag="pp")
                nc.tensor.matmul(pp[:], lhsT=adir_t[:],
                                 rhs=ta[:], start=True, stop=False)
                nc.tensor.matmul(pp[:], lhsT=apath_t[:],
                                 rhs=tb[:], start=False, stop=True)
                lp = work.tile([128, BS], f32r, tag="lp")
                nc.scalar.activation(lp[:], pp[:], AF.Exp)

                ev = tmpp.tile([128, C], f32, tag="ev")
                nc.scalar.activation(ev[:], ll_t[:, i * C:(i + 1) * C], AF.Exp)
                sv = tmpp.tile([128, 1], f32, tag="sv")
                nc.vector.tensor_reduce(out=sv[:], in_=ev[:], op=ALU.add,
                                        axis=AX.X)
                rv = tmpp.tile([128, 1], f32, tag="rv")
                nc.vector.reciprocal(rv[:], sv[:])
                vt = work.tile([128, C], f32r, tag="vt")
                nc.vector.tensor_scalar(out=vt[:], in0=ev[:], scalar1=rv[:, 0:1],
                                        scalar2=w2c[:, i:i + 1],
                                        op0=ALU.mult, op1=ALU.mult)
                nc.tensor.matmul(out_ps[:], lhsT=vt[:],
                                 rhs=lp[:],
                                 start=(i == 0), stop=(i == NTILES - 1))

            out_sb = work.tile([C, BS], f32, tag="osb")
            nc.vector.tensor_copy(out=out_sb[:], in_=out_ps[:])
            nc.sync.dma_start(out=out[:], in_=out_sb[:])

    nc.finalize()
    return nc


def _get_nc():
    if "nc" not in _NC_CACHE:
        _NC_CACHE["nc"] = _build_bass()
    return _NC_CACHE["nc"]


def kernel(x, split_weights, split_bias, leaf_logits, tree_weights):
    from concourse.bass_utils import run_bass_kernel_spmd

    x = np.ascontiguousarray(np.asarray(x, np.float32))
    split_weights = np.asarray(split_weights, np.float32)
    split_bias = np.asarray(split_bias, np.float32)
    leaf_logits = np.asarray(leaf_logits, np.float32)
    tree_weights = np.asarray(tree_weights, np.float32)

    adir, apath, a2, e2 = _host_consts()

    wpad = np.zeros((T, NPAD, D), np.float32)
    wpad[:, :N, :] = split_weights
    wtT = np.ascontiguousarray(wpad.reshape(TNP, D).T)          # [D, TNP]
    bpad = np.zeros((T, NPAD), np.float32)
    bpad[:, :N] = split_bias
    biasc = np.ascontiguousarray(bpad.reshape(NTILES, 128).T)   # [128, NTILES]
    llf = np.ascontiguousarray(leaf_logits.reshape(TNP, C))
    tw = np.ascontiguousarray(tree_weights.reshape(1, T))

    shared = dict(wt=wtT, biasc=biasc, llf=llf, tw=tw,
                  adir=adir, apath=apath, a2=a2, e2=e2)
    in_maps = []
    for i in range(NCORES):
        xt = np.ascontiguousarray(x[i * BS:(i + 1) * BS, :].T)  # [D, BS]
        in_maps.append(dict(xt=xt, **shared))

    nc = _get_nc()
    res = run_bass_kernel_spmd(nc, in_maps, core_ids=list(range(NCORES)))
    out = np.concatenate([res.results[i]["out"] for i in range(NCORES)],
                         axis=1).T                              # [B, C]
    return np.ascontiguousarray(out.astype(np.float32))


# revision 4
# speedup vs baseline: 1.5963x; 1.5963x over previous
"""Trainium2 Bass kernel for the soft-decision-tree ensemble classifier.

Math (per batch row b, tree t):
  zb[t,n]      = x[b] . W[t,n] + bias[t,n]
  log s        = zb - softplus(zb);  log(1-s) = -softplus(zb)
  log_leaf[l]  = sum_{k in path(l)} dir_k * zb_k  -  sum_{k in path(l)} softplus(zb_k)
  leaf_prob    = exp(log_leaf)
  out[b,c]     = sum_t 2*softmax(tw)_t * sum_l leaf_prob[t,l] * softmax(leaf_logits[t,l])_c

Mapping: data-parallel over the batch (B=4096 -> 512 rows per NeuronCore).
Per core, logits live in [tree-node (padded 64/tree), batch] layout so the
per-tree path sums become 128-wide matmuls with +/-1 constant matrices
(block-diagonal over a pair of trees per 128-partition tile). All matmuls
run as float32r (full PE rate at free-dim 512). softplus = Ln(Exp(x)+1) so
the whole kernel uses a single ACT function table (exp/ln).
"""

import numpy as np

TREE_DEPTH = 6
T, N, D, C = 64, 63, 512, 100
L = 2**TREE_DEPTH          # 64
NPAD = 64                  # nodes padded per tree
TNP = T * NPAD             # 4096
NTILES = TNP // 128        # 32 (two trees per 128-partition tile)
B = 4096
NCORES = 8
BS = B // NCORES           # 512


def _leaf_paths(depth):
    Ll = 2**depth
    idx = np.zeros((Ll, depth), np.int32)
    dr = np.zeros((Ll, depth), np.int32)
    for l in range(Ll):
        node = 0
        for k in range(depth):
            bit = (l >> (depth - 1 - k)) & 1
            idx[l, k] = node
            dr[l, k] = bit
            node = 2 * node + 1 + bit
    return idx, dr


def _host_consts():
    idx, dr = _leaf_paths(TREE_DEPTH)
    mdir = np.zeros((NPAD, L), np.float32)   # [node, leaf] +1 where dir=1
    mpath = np.zeros((NPAD, L), np.float32)  # [node, leaf] -1 on path
    for l in range(L):
        for k in range(TREE_DEPTH):
            n = idx[l, k]
            mpath[n, l] -= 1.0
            if dr[l, k]:
                mdir[n, l] += 1.0
    adir = np.zeros((128, 128), np.float32)
    apath = np.zeros((128, 128), np.float32)
    adir[:NPAD, :L] = mdir
    adir[NPAD:, L:] = mdir
    apath[:NPAD, :L] = mpath
    apath[NPAD:, L:] = mpath
    # a2[t, p] = 1 if (t % 2) == p // 64 ; e2[t, i] = 1 if t // 2 == i
    a2 = np.zeros((T, 128), np.float32)
    a2[0::2, :64] = 0.0
    for t in range(T):
        a2[t, (t % 2) * 64:(t % 2) * 64 + 64] = 1.0
    e2 = np.zeros((T, NTILES), np.float32)
    for t in range(T):
        e2[t, t // 2] = 1.0
    return adir, apath, a2, e2


_NC_CACHE = {}


def _build_bass():
    import concourse.bacc as bacc
    import concourse.mybir as mybir
    import concourse.tile as tile
    from concourse.masks import make_identity

    dt = mybir.dt
    f32 = dt.float32
    f32r = dt.float32r
    AF = mybir.ActivationFunctionType
    ALU = mybir.AluOpType
    AX = mybir.AxisListType

    nc = bacc.Bacc("TRN2", target_bir_lowering=False, debug=False,
                   num_devices=NCORES)

    # Pin the ACT function table to one containing BOTH Exp and Ln, else the
    # table-load pass ping-pongs between single-function tables (~1.3us per
    # reload, one per activation).
    from concourse.hw_specs import get_activation_tables
    AFT = mybir.ActivationFunctionType
    table_id = next(i for i, (_, funcs) in
                    enumerate(get_activation_tables("gen3").items())
                    if AFT.Exp in funcs and AFT.Ln in funcs)
    nc.scalar.add_instruction(mybir.InstLoadActFuncSet(
        name=f"I-{nc.next_id()}", ins=[], outs=[], act_func_set_id=table_id))

    xt = nc.dram_tensor("xt", [D, BS], f32r, kind="ExternalInput").ap()
    wt = nc.dram_tensor("wt", [D, TNP], f32r, kind="ExternalInput").ap()
    biasc = nc.dram_tensor("biasc", [128, NTILES], f32, kind="ExternalInput").ap()
    llf = nc.dram_tensor("llf", [TNP, C], f32, kind="ExternalInput").ap()
    tw = nc.dram_tensor("tw", [1, T], f32, kind="ExternalInput").ap()
    adir = nc.dram_tensor("adir", [128, 128], f32r, kind="ExternalInput").ap()
    apath = nc.dram_tensor("apath", [128, 128], f32r, kind="ExternalInput").ap()
    a2 = nc.dram_tensor("a2", [T, 128], f32, kind="ExternalInput").ap()
    e2 = nc.dram_tensor("e2", [T, NTILES], f32, kind="ExternalInput").ap()
    out = nc.dram_tensor("out", [C, BS], f32, kind="ExternalOutput").ap()

    with tile.TileContext(nc) as tc:
        with (
            tc.tile_pool(name="big", bufs=1) as bigp,
            tc.tile_pool(name="const", bufs=1) as constp,
            tc.tile_pool(name="work", bufs=3) as work,
            tc.tile_pool(name="tmp", bufs=2) as tmpp,
            tc.tile_pool(name="ps", bufs=2, space="PSUM") as psp,
            tc.tile_pool(name="ps1", bufs=1, space="PSUM") as ps1,
        ):
            # ---- persistent loads -------------------------------------
            wt_t = []
            for j in range(4):
                wtile = bigp.tile([128, TNP], f32r, tag=f"wt{j}")
                nc.sync.dma_start(out=wtile[:], in_=wt[j * 128:(j + 1) * 128, :])
                wt_t.append(wtile)
            xt_t = []
            for j in range(4):
                xtile = bigp.tile([128, BS], f32r, tag=f"xt{j}")
                nc.sync.dma_start(out=xtile[:], in_=xt[j * 128:(j + 1) * 128, :])
                xt_t.append(xtile)
            ll_t = bigp.tile([128, NTILES * C], f32, tag="ll")
            nc.sync.dma_start(
                out=ll_t[:].rearrange("p (i c) -> p i c", c=C),
                in_=llf.rearrange("(i p) c -> p i c", p=128),
            )
            biasc_t = constp.tile([128, NTILES], f32, tag="biasc")
            nc.sync.dma_start(out=biasc_t[:], in_=biasc[:])
            adir_t = constp.tile([128, 128], f32r, tag="adir")
            nc.sync.dma_start(out=adir_t[:], in_=adir[:])
            apath_t = constp.tile([128, 128], f32r, tag="apath")
            nc.sync.dma_start(out=apath_t[:], in_=apath[:])
            a2_t = constp.tile([T, 128], f32, tag="a2")
            nc.sync.dma_start(out=a2_t[:], in_=a2[:])
            e2_t = constp.tile([T, NTILES], f32, tag="e2")
            nc.sync.dma_start(out=e2_t[:], in_=e2[:])
            tw_t = constp.tile([1, T], f32, tag="tw")
            nc.sync.dma_start(out=tw_t[:], in_=tw[:])

            ident = constp.tile([64, 64], f32, tag="ident")
            make_identity(nc, ident[:])

            # ---- tree-weight softmax -> per-partition scale columns ----
            mneg = constp.tile([1, 1], f32, tag="mneg")
            nc.vector.tensor_reduce(out=mneg[:], in_=tw_t[:], op=ALU.max,
                                    axis=AX.X, negate=True)
            ew = constp.tile([1, T], f32, tag="ew")
            nc.scalar.activation(ew[:], tw_t[:], AF.Exp, bias=mneg[:, 0:1],
                                 scale=1.0)
            sw = constp.tile([1, 1], f32, tag="sw")
            nc.vector.tensor_reduce(out=sw[:], in_=ew[:], op=ALU.add, axis=AX.X)
            rw = constp.tile([1, 1], f32, tag="rw")
            nc.vector.reciprocal(rw[:], sw[:])
            wrow = constp.tile([1, T], f32, tag="wrow")
            nc.vector.tensor_scalar(out=wrow[:], in0=ew[:], scalar1=rw[:, 0:1],
                                    scalar2=2.0, op0=ALU.mult, op1=ALU.mult)
            wcol_ps = ps1.tile([T, 1], f32, tag="wcolps")
            nc.tensor.transpose(wcol_ps[:], wrow[:], ident[0:1, 0:1])
            wcol = constp.tile([T, 1], f32, tag="wcol")
            nc.vector.tensor_copy(out=wcol[:], in_=wcol_ps[:])
            bmat = constp.tile([T, NTILES], f32, tag="bmat")
            nc.vector.tensor_scalar_mul(bmat[:], e2_t[:], wcol[:, 0:1])
            w2_ps = ps1.tile([128, NTILES], f32, tag="w2ps")
            nc.tensor.matmul(w2_ps[:], lhsT=a2_t[:], rhs=bmat[:],
                             start=True, stop=True)
            w2c = constp.tile([128, NTILES], f32, tag="w2c")
            nc.vector.tensor_copy(out=w2c[:], in_=w2_ps[:])

            # ---- main pipeline ---------------------------------------
            out_ps = ps1.tile([C, BS], f32, tag="outps")
            for i in range(NTILES):
                pz = psp.tile([128, BS], f32, tag="pz")
                for j in range(4):
                    nc.tensor.matmul(
                        pz[:],
                        lhsT=wt_t[j][:, i * 128:(i + 1) * 128],
                        rhs=xt_t[j][:],
                        start=(j == 0), stop=(j == 3),
                    )
                bsl = biasc_t[:, i:i + 1]
                ta = work.tile([128, BS], f32r, tag="ta")
                nc.vector.tensor_scalar_add(out=ta[:], in0=pz[:], scalar1=bsl)
                te = tmpp.tile([128, BS], f32, tag="te")
                nc.scalar.activation(te[:], pz[:], AF.Exp, bias=bsl, scale=1.0)
                tb = work.tile([128, BS], f32r, tag="tb")
                nc.scalar.activation(tb[:], te[:], AF.Ln, bias=1.0, scale=1.0)
                pp = psp.tile([128, BS], f32, tag="pp")
                nc.tensor.matmul(pp[:], lhsT=adir_t[:],
                                 rhs=ta[:], start=True, stop=False)
                nc.tensor.matmul(pp[:], lhsT=apath_t[:],
                                 rhs=tb[:], start=False, stop=True)
                lp = work.tile([128, BS], f32r, tag="lp")
                nc.scalar.activation(lp[:], pp[:], AF.Exp)

                ev = tmpp.tile([128, C], f32, tag="ev")
                nc.scalar.activation(ev[:], ll_t[:, i * C:(i + 1) * C], AF.Exp)
                sv = tmpp.tile([128, 1], f32, tag="sv")
                nc.vector.tensor_reduce(out=sv[:], in_=ev[:], op=ALU.add,
                                        axis=AX.X)
                rv = tmpp.tile([128, 1], f32, tag="rv")
                nc.vector.reciprocal(rv[:], sv[:])
                vt = work.tile([128, C], f32r, tag="vt")
                nc.vector.tensor_scalar(out=vt[:], in0=ev[:], scalar1=rv[:, 0:1],
                                        scalar2=w2c[:, i:i + 1],
                                        op0=ALU.mult, op1=ALU.mult)
                nc.tensor.matmul(out_ps[:], lhsT=vt[:],
                                 rhs=lp[:],
                                 start=(i == 0), stop=(i == NTILES - 1))

            out_sb = work.tile([C, BS], f32, tag="osb")
            nc.vector.tensor_copy(out=out_sb[:], in_=out_ps[:])
            nc.sync.dma_start(out=out[:], in_=out_sb[:])

    nc.finalize()
    return nc


def _get_nc():
    if "nc" not in _NC_CACHE:
        _NC_CACHE["nc"] = _build_bass()
    return _NC_CACHE["nc"]


def kernel(x, split_weights, split_bias, leaf_logits, tree_weights):
    from concourse.bass_utils import run_bass_kernel_spmd

    x = np.ascontiguousarray(np.asarray(x, np.float32))
    split_weights = np.asarray(split_weights, np.float32)
    split_bias = np.asarray(split_bias, np.float32)
    leaf_logits = np.asarray(leaf_logits, np.float32)
    tree_weights = np.asarray(tree_weights, np.float32)

    adir, apath, a2, e2 = _host_consts()

    wpad = np.zeros((T, NPAD, D), np.float32)
    wpad[:, :N, :] = split_weights
    wtT = np.ascontiguousarray(wpad.reshape(TNP, D).T)          # [D, TNP]
    bpad = np.zeros((T, NPAD), np.float32)
    bpad[:, :N] = split_bias
    biasc = np.ascontiguousarray(bpad.reshape(NTILES, 128).T)   # [128, NTILES]
    llf = np.ascontiguousarray(leaf_logits.reshape(TNP, C))
    tw = np.ascontiguousarray(tree_weights.reshape(1, T))

    shared = dict(wt=wtT, biasc=biasc, llf=llf, tw=tw,
                  adir=adir, apath=apath, a2=a2, e2=e2)
    in_maps = []
    for i in range(NCORES):
        xt = np.ascontiguousarray(x[i * BS:(i + 1) * BS, :].T)  # [D, BS]
        in_maps.append(dict(xt=xt, **shared))

    nc = _get_nc()
    res = run_bass_kernel_spmd(nc, in_maps, core_ids=list(range(NCORES)))
    out = np.concatenate([res.results[i]["out"] for i in range(NCORES)],
                         axis=1).T                              # [B, C]
    return np.ascontiguousarray(out.astype(np.float32))
